# revision 40
# baseline (speedup 1.0000x reference)
"""Trainium2 Bass kernel for batched multi-head attention (B=8, N=M=C=1024,
H=16, D=64), data-parallel across 8 NeuronCores (one batch element per core).

v2: host-side prep of all layouts (pre-transposed bf16 q/k/v/target-mask and
weights; key mask folded into v), so the device kernel is pure compute:

Per-core dataflow (bf16 matmul inputs, f32 PSUM accumulate):
  - per head-pair j: project qh^T/kh^T (head-transposed) from pre-staged
    qbT/kbT; vh (natural, key-masked on host) with a trailing "key-indicator"
    column so the AV matmul also produces the softmax denominator.
  - QK^T row-packed two heads at a time (disjoint PE row groups, separate
    PSUM banks); exp on the scalar engine straight from PSUM with 1/sqrt(D)
    folded in; target mask as [128,512] bf16 DVE multiplies; AV with
    65-column lhsT -> numerator + denominator in one accumulation chain.
  - normalize via K=1 f32r ones-matmul broadcast of the denominator row +
    fast reciprocal + multiply; o-projection as K=128 accumulation chains
    with bo folded in as a K=1 ones matmul.
  - the PE instruction stream is software-pipelined: projection chains for
    head-pair j+1 are emitted between the QK^T groups of pair j, so the PE
    has filler work while the scalar engine computes exp (the scalar engine
    needs ~2x longer per score tile than the PE).
"""
import sys

sys.path.insert(0, "/opt/trn_rl_repo")

import numpy as np
import ml_dtypes

import concourse.bass as bass  # noqa: F401
import concourse.mybir as mybir
import concourse.bacc as bacc
import concourse.tile as tile
from concourse import bass_utils

B = 8
N = 1024   # queries
M = 1024   # keys
C = 1024   # model dim
H = 16
D = 64
NP = 8     # head pairs
P = 128
NB = 2     # n blocks of 512
SCALE = D ** -0.5

F32 = mybir.dt.float32
F32R = mybir.dt.float32r
BF16 = mybir.dt.bfloat16
I32 = mybir.dt.int32
MUL = mybir.AluOpType.mult
EXP = mybir.ActivationFunctionType.Exp
NPBF = ml_dtypes.bfloat16

_NC_CACHE = {}


def build_nc():
    nc = bacc.Bacc("TRN2", target_bir_lowering=False, debug=False, num_devices=1)

    qbT_d = nc.dram_tensor("qbT", [P, 8, N], BF16, kind="ExternalInput").ap()
    kbT_d = nc.dram_tensor("kbT", [P, 8, M], BF16, kind="ExternalInput").ap()
    vbT_d = nc.dram_tensor("vbT", [P, 8, M], BF16, kind="ExternalInput").ap()
    tmT_d = nc.dram_tensor("tmT", [P, 8, N], BF16, kind="ExternalInput").ap()
    mb_d = nc.dram_tensor("maskb", [P, 8], BF16, kind="ExternalInput").ap()
    wq_d = nc.dram_tensor("wq", [NP, P, 8, P], BF16, kind="ExternalInput").ap()
    wk_d = nc.dram_tensor("wk", [NP, P, 8, P], BF16, kind="ExternalInput").ap()
    wv_d = nc.dram_tensor("wv", [4, P, 8, 256], BF16, kind="ExternalInput").ap()
    wo_d = nc.dram_tensor("wo", [NP, P, C], BF16, kind="ExternalInput").ap()
    bob_d = nc.dram_tensor("bob", [P, C], BF16, kind="ExternalInput").ap()
    out_d = nc.dram_tensor("out", [N, C], F32, kind="ExternalOutput").ap()

    with tile.TileContext(nc) as tc:
        _body(tc, nc, qbT_d, kbT_d, vbT_d, tmT_d, mb_d, wq_d, wk_d, wv_d,
              wo_d, bob_d, out_d)
    nc.compile()
    return nc


def _body(tc, nc, qbT_d, kbT_d, vbT_d, tmT_d, mb_d, wq_d, wk_d, wv_d, wo_d,
          bob_d, out_d):
    from contextlib import ExitStack
    ctx = ExitStack()
    with ctx:
        persist = ctx.enter_context(tc.tile_pool(name="persist", bufs=1))
        wpool = ctx.enter_context(tc.tile_pool(name="wpool", bufs=2))
        ptpool = ctx.enter_context(tc.tile_pool(name="ptpool", bufs=8))
        xpool = ctx.enter_context(tc.tile_pool(name="xpool", bufs=2))
        opool = ctx.enter_context(tc.tile_pool(name="opool", bufs=2))
        spsum = ctx.enter_context(tc.tile_pool(name="spsum", bufs=2, space="PSUM"))
        avpsum = ctx.enter_context(tc.tile_pool(name="avpsum", bufs=2, space="PSUM"))

        # ---- persistent SBUF tensors ----
        qbT = persist.tile([P, 8, N], BF16)   # [p, cc, n] = q[n, cc*128+p]
        kbT = persist.tile([P, 8, M], BF16)
        vbT = persist.tile([P, 8, M], BF16)   # key-masked v, transposed
        tmT = persist.tile([P, 8, N], BF16)   # [p, mc, n] = tmask[n, mc*128+p]
        qhT = persist.tile([P, NP, N], BF16)  # [p, j, n] = qh[n, j*128+p]
        # khTz[:, j, 0, m]: rows 0:64 = kh^T head 2j, rows 64:128 = 0
        # khTz[:, j, 1, m]: rows 0:64 = 0, rows 64:128 = kh^T head 2j+1
        # Zero-padding makes every QK^T matmul a uniform K=128 config (same
        # PE tile config as the projections), avoiding per-instruction
        # reconfig between interleaved chains. Zero quadrants are filled
        # per-(j, nb) in the k-projection evacuation, not in one big memset
        # (a 16K-row DVE memset would stall the vector queue at startup).
        khTz = persist.tile([P, NP, 2, M], BF16)
        vha = persist.tile([P, NP, 8, 130], BF16)
        xn = persist.tile([P, NP, N], BF16)   # [p, j, n] = x_norm[n, j*128+p]
        wob = persist.tile([P, NP, C], BF16)  # [p, j, c2] = Wo[j*128+p, c2]
        maskb = persist.tile([P, 8], BF16)
        bob = persist.tile([P, C], BF16)  # row 0 = bias, rows 1.. = 0
        # ones-row constants: lhsT of K-padded broadcast matmuls. Using a
        # 128-row lhsT keeps every matmul in the kernel at the same
        # (128, 128) PE tile config -- a config switch costs a ~770ns
        # pipeline flush.
        onesr0 = persist.tile([P, P], BF16)   # row 0 = 1, rest 0
        nc.vector.memset(onesr0[:], 0.0)
        nc.vector.memset(onesr0[0:1, :], 1.0)
        onesr64 = persist.tile([P, P], BF16)  # row 64 = 1, rest 0
        nc.vector.memset(onesr64[:], 0.0)
        nc.vector.memset(onesr64[64:65, :], 1.0)

        # ---- input DMAs (split into ~128KB chunks across queues) ----
        nc.sync.dma_start(out=maskb[:], in_=mb_d)
        nc.sync.dma_start(out=bob[:], in_=bob_d)

        def load_weights(j, split=1):
            wqb = wpool.tile([P, 8, P], BF16, tag="wq")
            wkb = wpool.tile([P, 8, P], BF16, tag="wk")
            s = 8 // split
            for i in range(split):
                cs = slice(i * s, (i + 1) * s)
                nc.sync.dma_start(out=wqb[:, cs, :], in_=wq_d[j, :, cs, :])
                nc.sync.dma_start(out=wkb[:, cs, :], in_=wk_d[j, :, cs, :])
            wvb = None
            if j % 2 == 0:
                wvb = wpool.tile([P, 8, 256], BF16, tag="wv")
                for i in range(4):
                    cs = slice(i * 2, (i + 1) * 2)
                    nc.sync.dma_start(out=wvb[:, cs, :], in_=wv_d[j // 2, :, cs, :])
            nc.sync.dma_start(out=wob[:, j, :], in_=wo_d[j])
            return wqb, wkb, wvb

        # PE warmup: keep the clock domain hot while the first DMAs land.
        wu = persist.tile([P, 512], BF16)
        nc.vector.memset(wu[:], 0.0)
        wups = spsum.tile([P, 2, 512], F32, tag="sp", name="wups")
        for i in range(24):
            nc.tensor.matmul(wups[:, i % 2, :], wu[:, 0:P], wu[:],
                             start=True, stop=True)

        wqb0, wkb0, wvb0 = load_weights(0, split=8)
        # DMA priority order = consumption order of the first iteration:
        # proj(0) nb0 operands, then the nb0 target-mask (needed by the
        # first DVE multiplies ~12us in), then v/wv for AV(0), then the
        # nb1 halves. Issue alternates between the sync and gpsimd
        # sequencers: a single sequencer takes ~565ns per dma_start, which
        # would serialize ~60 descriptors into ~35us of issue latency.
        startup = []
        for cc in range(8):
            for h in range(2):
                cs = slice(h * 256, h * 256 + 256)
                startup.append((qbT[:, cc, cs], qbT_d[:, cc, cs]))
            for h in range(2):
                cs = slice(h * 256, h * 256 + 256)
                startup.append((kbT[:, cc, cs], kbT_d[:, cc, cs]))
        for mc in range(8):
            startup.append((tmT[:, mc, 0:512], tmT_d[:, mc, 0:512]))
        for cc in range(8):
            startup.append((vbT[:, cc, :], vbT_d[:, cc, :]))
        for cc in range(8):
            startup.append((qbT[:, cc, 512:1024], qbT_d[:, cc, 512:1024]))
            startup.append((kbT[:, cc, 512:1024], kbT_d[:, cc, 512:1024]))
        for mc in range(8):
            startup.append((tmT[:, mc, 512:1024], tmT_d[:, mc, 512:1024]))
        for i, (dst, src) in enumerate(startup):
            eng = nc.sync if i % 2 == 0 else nc.gpsimd
            eng.dma_start(out=dst, in_=src)

        # ---- helpers ----
        def make_proj_fillers(j, wqb, wkb):
            tiles = {}

            def chain(which, nb):
                def run():
                    if nb not in tiles:
                        tiles[nb] = spsum.tile([P, 2, 512], F32, tag="sp",
                                               name=f"pj{j}_{nb}")
                    sp = tiles[nb]
                    ns = slice(nb * 512, (nb + 1) * 512)
                    half = 0 if which == "q" else 1
                    wb = wqb if which == "q" else wkb
                    src = qbT if which == "q" else kbT
                    for cc in range(8):
                        nc.tensor.matmul(sp[:, half, :], wb[:, cc, :],
                                         src[:, cc, ns],
                                         start=(cc == 0), stop=(cc == 7))
                    if which == "q":
                        nc.vector.tensor_copy(qhT[:, j, ns], sp[:, half, :])
                    else:
                        if nb == 0:
                            nc.gpsimd.memset(khTz[64:128, j, 0, :], 0.0)
                            nc.gpsimd.memset(khTz[0:64, j, 1, :], 0.0)
                        nc.vector.tensor_copy(khTz[0:64, j, 0, ns],
                                              sp[0:64, half, :])
                        nc.vector.tensor_copy(khTz[64:128, j, 1, ns],
                                              sp[64:128, half, :])
                return run

            return [chain("q", 0), chain("k", 0), chain("q", 1), chain("k", 1)]

        def make_vproj_fillers(j, wvb):
            """v projection chains for pair (j, j+1), one filler per 2 m-chunks."""
            tiles = {}

            def chain(mg, mi2):
                def run():
                    if mg not in tiles:
                        tiles[mg] = spsum.tile([P, 2, 512], F32, tag="sp",
                                               name=f"pv{j}_{mg}")
                    pvv = tiles[mg].rearrange("p a b -> p (a b)").rearrange(
                        "p (m d) -> p m d", m=4)
                    for mi in (mi2, mi2 + 1):
                        mc = mg * 4 + mi
                        ms = slice(mc * P, (mc + 1) * P)
                        for cc in range(8):
                            nc.tensor.matmul(pvv[:, mi, :], vbT[:, cc, ms],
                                             wvb[:, cc, :],
                                             start=(cc == 0), stop=(cc == 7))
                        out_sl = vha[:, j:j + 2, mc, :].rearrange(
                            "p j (hx dd) -> p j hx dd", hx=2)[:, :, :, 0:64]
                        in_sl = pvv[:, mi, :].rearrange(
                            "p (j hx dd) -> p j hx dd", j=2, hx=2)
                        nc.vector.tensor_copy(out_sl, in_sl)
                    if mg == 1 and mi2 == 2:
                        for jx in (j, j + 1):
                            nc.vector.tensor_copy(vha[:, jx, :, 64], maskb[:])
                            nc.vector.tensor_copy(vha[:, jx, :, 129], maskb[:])
                return run

            return [chain(0, 0), chain(0, 2), chain(1, 0), chain(1, 2)]

        def qk_attn_g(j, nb, g, ptiles):
            """One QK^T group: scores for 2 m-chunks x 2 heads, exp, tm-mask."""
            ns = slice(nb * 512, (nb + 1) * 512)
            sp0 = spsum.tile([P, 2, 512], F32, tag="sp")
            sp1 = spsum.tile([P, 2, 512], F32, tag="sp")
            for mcx in range(2):
                mc = 2 * g + mcx
                ms = slice(mc * P, (mc + 1) * P)
                nc.tensor.matmul(sp0[:, mcx, :], khTz[:, j, 0, ms],
                                 qhT[:, j, ns], start=True, stop=True)
            for mcx in range(2):
                mc = 2 * g + mcx
                ms = slice(mc * P, (mc + 1) * P)
                nc.tensor.matmul(sp1[:, mcx, :], khTz[:, j, 1, ms],
                                 qhT[:, j, ns], start=True, stop=True)
            pt0 = ptpool.tile([P, 2, 512], BF16, tag="pt")
            pt1 = ptpool.tile([P, 2, 512], BF16, tag="pt")
            nc.scalar.activation(pt0[:], sp0[:], EXP, scale=SCALE)
            nc.scalar.activation(pt1[:], sp1[:], EXP, scale=SCALE)
            for mcx in range(2):
                mc = 2 * g + mcx
                tsl = tmT[:, mc, ns]
                nc.vector.tensor_tensor(pt0[:, mcx, :], pt0[:, mcx, :], tsl, MUL)
                nc.vector.tensor_tensor(pt1[:, mcx, :], pt1[:, mcx, :], tsl, MUL)
            ptiles[0][g] = pt0
            ptiles[1][g] = pt1

        av_tiles = {}

        def av_attn_half(j, nb, ptiles, half):
            """AV numerator+denominator accumulation, m-chunks half*4..half*4+3."""
            if half == 0:
                av_tiles[nb] = (avpsum.tile([65, 512], F32, tag="av", name="av0"),
                                avpsum.tile([65, 512], F32, tag="av", name="av1"))
            av0, av1 = av_tiles[nb]
            for mc in range(half * 4, half * 4 + 4):
                g, mcx = mc // 2, mc % 2
                nc.tensor.matmul(av0[:], vha[:, j, mc, 0:65],
                                 ptiles[0][g][:, mcx, :],
                                 start=(mc == 0), stop=(mc == 7))
            for mc in range(half * 4, half * 4 + 4):
                g, mcx = mc // 2, mc % 2
                nc.tensor.matmul(av1[:], vha[:, j, mc, 65:130],
                                 ptiles[1][g][:, mcx, :],
                                 start=(mc == 0), stop=(mc == 7))

        def norm_den(nb):
            """Phase 1: denominator rows -> bf16 SBUF (scalar engine),
            emitted as soon as the AV chains finish so the copy clears the
            scalar queue before the bc matmul needs it."""
            av0, av1 = av_tiles[nb]
            dens = []
            for av in (av0, av1):
                denb = xpool.tile([65, 512], BF16, tag="denb", bufs=4)
                nc.scalar.copy(denb[:], av[:])
                dens.append(denb)
            return dens

        def norm_fin(j, nb, dens):
            """Phase 2: K=1 ones-matmul broadcast + reciprocal + multiply."""
            ns = slice(nb * 512, (nb + 1) * 512)
            av0, av1 = av_tiles[nb]
            for hx, (av, denb) in enumerate(((av0, dens[0]), (av1, dens[1]))):
                bc = avpsum.tile([P, 512], F32, tag="bc", bufs=2)
                nc.tensor.matmul(bc[:], onesr64[0:65, :], denb[:],
                                 start=True, stop=True)
                rc = xpool.tile([64, 512], F32, tag="rc")
                nc.vector.reciprocal_approx_fast(rc[:], bc[0:64, :])
                rows = slice(0, 64) if hx == 0 else slice(64, 128)
                nc.vector.tensor_tensor(xn[rows, j, ns], av[0:64, :], rc[:], MUL)

        def oproj_chain(nch, c2h):
            nsl = slice(nch * P, (nch + 1) * P)
            c2s = slice(c2h * 512, (c2h + 1) * 512)
            po = spsum.tile([P, 2, 512], F32, tag="sp", name="po")
            nc.tensor.matmul(po[:, 0, :], onesr0[:], bob[:, c2s],
                             start=True, stop=False)
            for jj in range(NP):
                nc.tensor.matmul(po[:, 0, :], xn[:, jj, nsl], wob[:, jj, c2s],
                                 start=False, stop=(jj == NP - 1))
            ot = opool.tile([P, 512], F32, tag="ot")
            nc.scalar.copy(ot[:], po[:, 0, :])
            nc.sync.dma_start(out=out_d[nsl, c2s], in_=ot[:])

        # ---- software-pipelined main loop ----
        # j=0's projections are the only pre-loop work; v-proj(0) runs as
        # j=0's fillers so the PE does not stall on the (later-arriving)
        # vbT DMA before the first QK groups.
        for f in make_proj_fillers(0, wqb0, wkb0):
            f()

        pending = None  # (j, dens) for the deferred nb1 normalize
        for j in range(NP):
            fillers = []
            if j == 0:
                fillers += make_vproj_fillers(0, wvb0)
            if j + 1 < NP:
                wqb_n, wkb_n, wvb_n = load_weights(j + 1, split=4)
                fillers += make_proj_fillers(j + 1, wqb_n, wkb_n)
                if (j + 1) % 2 == 0:
                    fillers += make_vproj_fillers(j + 1, wvb_n)
            fi = 0

            pt_nb0 = [[None] * 4, [None] * 4]
            for g in range(4):
                qk_attn_g(j, 0, g, pt_nb0)
                if g == 0 and pending is not None:
                    norm_fin(pending[0], 1, pending[1])
                    pending = None
                if fi < len(fillers):
                    fillers[fi]()
                    fi += 1
            pt_nb1 = [[None] * 4, [None] * 4]
            qk_attn_g(j, 1, 0, pt_nb1)
            av_attn_half(j, 0, pt_nb0, 0)
            qk_attn_g(j, 1, 1, pt_nb1)
            av_attn_half(j, 0, pt_nb0, 1)
            dens0 = norm_den(0)
            qk_attn_g(j, 1, 2, pt_nb1)
            norm_fin(j, 0, dens0)
            if fi < len(fillers):
                fillers[fi]()
                fi += 1
            qk_attn_g(j, 1, 3, pt_nb1)
            while fi < len(fillers):
                fillers[fi]()
                fi += 1
            if j == NP - 1:
                # o-projection chains over the nb0 query rows only need
                # norm_fin(7, 0) (already emitted): they fill the PE while
                # the scalar engine computes the last exp batch.
                for nch in range(4):
                    for c2h in range(2):
                        oproj_chain(nch, c2h)
            av_attn_half(j, 1, pt_nb1, 0)
            av_attn_half(j, 1, pt_nb1, 1)
            pending = (j, norm_den(1))

        # ---- o-projection tail (+ bias) ----
        if pending is not None:
            norm_fin(pending[0], 1, pending[1])
            pending = None
        for nch in range(4, 8):
            for c2h in range(2):
                oproj_chain(nch, c2h)


def _get_nc():
    if "nc" not in _NC_CACHE:
        _NC_CACHE["nc"] = build_nc()
    return _NC_CACHE["nc"]


def _prep_inputs(q, k, v, mask, target_mask, Wq, Wk, Wv, Wo, bo):
    """Host-side staging: transpose + bf16-cast into exact device layouts."""
    q = np.asarray(q, np.float32)
    k = np.asarray(k, np.float32)
    v = np.asarray(v, np.float32)
    mask = np.asarray(mask, np.int32)
    target_mask = np.asarray(target_mask, np.int32)

    def t_layout(x):
        # [N, C] -> [p, cc, n] with value x[n, cc*128+p]
        xT = np.ascontiguousarray(x.T).astype(NPBF)
        return np.ascontiguousarray(xT.reshape(8, P, -1).transpose(1, 0, 2))

    Wqb = np.asarray(Wq, np.float32).astype(NPBF)
    Wkb = np.asarray(Wk, np.float32).astype(NPBF)
    Wvb = np.asarray(Wv, np.float32).astype(NPBF)
    Wob = np.asarray(Wo, np.float32).astype(NPBF)
    shared = {
        # wq[j, p, cc, dj] = Wq[cc*128+p, j*128+dj]
        "wq": np.ascontiguousarray(
            Wqb.reshape(8, P, NP, P).transpose(2, 1, 0, 3)),
        "wk": np.ascontiguousarray(
            Wkb.reshape(8, P, NP, P).transpose(2, 1, 0, 3)),
        # wv[jp, p, cc, dd] = Wv[cc*128+p, jp*256+dd]
        "wv": np.ascontiguousarray(
            Wvb.reshape(8, P, 4, 256).transpose(2, 1, 0, 3)),
        # wo[j, p, c2] = Wo[j*128+p, c2]
        "wo": np.ascontiguousarray(Wob.reshape(NP, P, C)),
        "bob": np.ascontiguousarray(np.concatenate(
            [np.asarray(bo, np.float32).astype(NPBF).reshape(1, C),
             np.zeros((P - 1, C), NPBF)], axis=0)),
    }
    in_maps = []
    for b in range(B):
        vm = v[b] * mask[b].astype(np.float32)[:, None]
        m = {
            "qbT": t_layout(q[b]),
            "kbT": t_layout(k[b]),
            "vbT": t_layout(vm),
            "tmT": t_layout(target_mask[b].astype(np.float32)),
            "maskb": np.ascontiguousarray(
                mask[b].astype(np.float32).astype(NPBF).reshape(8, P).T),
        }
        m.update(shared)
        in_maps.append(m)
    return in_maps


def kernel(q, k, v, mask, target_mask, Wq, Wk, Wv, Wo, bo):
    nc = _get_nc()
    in_maps = _prep_inputs(q, k, v, mask, target_mask, Wq, Wk, Wv, Wo, bo)
    res = bass_utils.run_bass_kernel_spmd(nc, in_maps, core_ids=list(range(B)))
    out = np.stack([res.results[b]["out"] for b in range(B)], axis=0)
    return out.astype(np.float32)


def run_traced(q, k, v, mask, target_mask, Wq, Wk, Wv, Wo, bo, **trace_kwargs):
    """Like kernel() but with NTFF tracing; returns (out, BassKernelResults)."""
    nc = _get_nc()
    in_maps = _prep_inputs(q, k, v, mask, target_mask, Wq, Wk, Wv, Wo, bo)
    res = bass_utils.run_bass_kernel_spmd(nc, in_maps, core_ids=list(range(B)),
                                          trace=True, **trace_kwargs)
    out = np.stack([res.results[b]["out"] for b in range(B)], axis=0)
    return out.astype(np.float32), res
